# revision 3
# baseline (speedup 1.0000x reference)
"""Masked nearest-neighbor (AnchorTs2Vec e_an) Trainium2 kernel.

Problem: for e_actv [8192, 256] f32 and host ids [8192], compute
    d2[i,j] = |e_i|^2 + |e_j|^2 - 2 e_i.e_j   (masked BIG where host_i==host_j, incl. diag)
    idx[i]  = argmin_j d2[i,j]   (first index on ties, matching jnp.argmin)
    e_an    = e_actv[idx]
Returns (e_actv, e_ap, e_an) like the reference.

Distribution: rows sharded across 8 NeuronCores (1024 rows/core); the
column operand (all 8192 embeddings) is replicated to every core, so no
collective is needed.

Device computation per core (per 128-row tile):
  - One fused matmul chain computes the *masked* distance surrogate
        val[i,j] = sq_j - 2*G_ij + 32768*[host_i==host_j]
    directly in PSUM via an extended contraction dimension: an fp16
    hi/lo split of e_actv (G = Eh.Eh + Eh.El + El.Eh with exact
    power-of-two +-64 balancing of the cross terms to stay in fp16
    normal range; products are computed exactly by the PE, fp32
    accumulated => ~1e-5 abs error vs the ~1.8e-4 min argmin margin of
    this distance distribution), plus 3 fp16 splits of sq_j and a
    32768*onehot64(host) mask block. sq_i is omitted (constant per row,
    argmin-invariant); sqrt is omitted (monotone, argmin-invariant).
  - DVE prefix-min scan (tensor_tensor_scan) over the 16 PSUM column
    tiles, carry-chained -> running min r; gmin = r[:, -1].
  - ACT Sign(r - gmin) with accum_out: sum of [r_j > gmin] = exact
    first-occurrence argmin index. Cast int32, DMA out.

Host (numpy) does only input staging (fp16 split, transposes, one-hot)
and the final trivial row gather e_actv[idx].
"""

import numpy as np

import concourse.tile as tile
from concourse import bacc, mybir
from concourse.bass_utils import run_bass_kernel_spmd

N, D, H = 8192, 256, 64
N_CORES = 8
RPC = N // N_CORES          # rows per core
P = 128                     # partitions
RT = RPC // P               # row tiles per core (8)
TS = 512                    # col tile size (one PSUM bank)
CT = N // TS                # col tiles (16)
NCHUNK = 7                  # A/B sbuf chunks: h0 h1 hs0 hs1 ls0 ls1 extras
BIGM = 32768.0              # mask constant (fp16-safe, >> max masked-val gap)
SCALE = np.float32(64.0)    # exact power-of-two balancing for cross terms

f16 = np.float16

# chunk pairing for the extended contraction (a_chunk i pairs b_chunk i):
#   0,1: (-2Eh)^T    x Eh^T        -> -2 Eh.Eh
#   2,3: (-2Eh/64)^T x (El*64)^T   -> -2 Eh.El
#   4,5: (-2El*64)^T x (Eh/64)^T   -> -2 El.Eh
#   6:   extras: [1,1,1,onehot(host_i)] x [sq1,sq2,sq3, BIGM*onehot(host_j)]
PAIRS = [(0, 0), (1, 1), (2, 2), (3, 3), (4, 4), (5, 5), (6, 6)]

_compiled = None


def _build():
    nc = bacc.Bacc("TRN2", target_bir_lowering=False, debug=False,
                   num_devices=N_CORES)
    b_in = nc.dram_tensor("b_in", [P, NCHUNK * N], mybir.dt.float16,
                          kind="ExternalInput").ap()
    a_in = nc.dram_tensor("a_in", [P, RT * NCHUNK * P], mybir.dt.float16,
                          kind="ExternalInput").ap()
    out_idx = nc.dram_tensor("out_idx", [RPC], mybir.dt.int32,
                             kind="ExternalOutput").ap()

    with tile.TileContext(nc) as tc:
        with tc.tile_pool(name="bp", bufs=1) as bp, \
             tc.tile_pool(name="ap_", bufs=1) as ap_, \
             tc.tile_pool(name="rp", bufs=1) as rp, \
             tc.tile_pool(name="sp", bufs=1) as sp, \
             tc.tile_pool(name="small", bufs=2) as small, \
             tc.tile_pool(name="psum", bufs=8, space="PSUM") as pp:
            btile = bp.tile([P, NCHUNK * N], mybir.dt.float16, tag="b")
            nc.sync.dma_start(btile[:], b_in)
            atile = ap_.tile([P, RT * NCHUNK * P], mybir.dt.float16, tag="a")
            nc.sync.dma_start(atile[:], a_in)
            dummy = sp.tile([P, TS], mybir.dt.float32, tag="dummy")
            nc.vector.memset(dummy[:], 0.0)

            for rt in range(RT):
                r = rp.tile([P, N], mybir.dt.float32, tag="r")
                for q in range(CT // 4):
                    ps_tiles = []
                    for _pi in range(4):
                        ps_t = pp.tile([P, TS], mybir.dt.float32, tag="ps")
                        ps_tiles.append(ps_t)
                    for c, (ac, bc) in enumerate(PAIRS):
                        a_off = (rt * NCHUNK + ac) * P
                        lhsT = atile[:, a_off:a_off + P]
                        for ctq in range(4):
                            ct = q * 4 + ctq
                            rhs = btile[:, bc * N + ct * TS: bc * N + (ct + 1) * TS]
                            nc.tensor.matmul(ps_tiles[ctq][:], lhsT, rhs,
                                             start=(c == 0),
                                             stop=(c == len(PAIRS) - 1))
                    for ctq in range(4):
                        ct = q * 4 + ctq
                        init = BIGM * 2 if ct == 0 else r[:, ct * TS - 1: ct * TS]
                        nc.vector.tensor_tensor_scan(
                            r[:, ct * TS:(ct + 1) * TS], ps_tiles[ctq][:],
                            dummy[:], initial=init,
                            op0=mybir.AluOpType.min, op1=mybir.AluOpType.bypass)

                negmin = small.tile([P, 1], mybir.dt.float32, tag="negmin")
                nc.vector.tensor_scalar_mul(negmin[:], r[:, N - 1:N], -1.0)
                sgn = sp.tile([P, N], mybir.dt.bfloat16, tag="sgn")
                cnt = small.tile([P, 1], mybir.dt.float32, tag="cnt")
                nc.scalar.activation(sgn[:], r[:],
                                     mybir.ActivationFunctionType.Sign,
                                     bias=negmin[:, 0:1], scale=1.0,
                                     accum_out=cnt[:])
                cnt_i = small.tile([P, 1], mybir.dt.int32, tag="cnti")
                nc.vector.tensor_copy(cnt_i[:], cnt[:])
                nc.sync.dma_start(out_idx[rt * P:(rt + 1) * P], cnt_i[:, 0])

    nc.compile()
    return nc


def _prep_inputs(e_actv: np.ndarray, host: np.ndarray):
    e = np.ascontiguousarray(np.asarray(e_actv, dtype=np.float32))
    hostv = np.asarray(host).astype(np.int64)

    eh = e.astype(f16)
    ehf = eh.astype(np.float32)
    elf = e - ehf

    chunks_b = [
        eh,                           # b0/b1: Eh
        (elf * SCALE).astype(f16),    # b2/b3: El*64
        (ehf / SCALE).astype(f16),    # b4/b5: Eh/64
    ]
    chunks_a = [
        (-2.0 * ehf).astype(f16),          # a0/a1: -2Eh
        (-2.0 * ehf / SCALE).astype(f16),  # a2/a3: -2Eh/64
        (-2.0 * elf * SCALE).astype(f16),  # a4/a5: -2El*64
    ]

    sq = (e.astype(np.float64) * e.astype(np.float64)).sum(1)
    s1 = sq.astype(np.float32).astype(f16)
    r1 = (sq - s1.astype(np.float64)).astype(np.float32)
    s2 = r1.astype(f16)
    s3 = (r1 - s2.astype(np.float32)).astype(f16)

    onehot = np.zeros((N, H), dtype=np.float32)
    onehot[np.arange(N), hostv] = 1.0

    # B side (replicated): 7 chunks [128, 8192] packed [128, 7*8192]
    b_all = np.zeros((P, NCHUNK * N), dtype=f16)
    for k, cb in enumerate(chunks_b):
        cbT = np.ascontiguousarray(cb.T)       # [256, 8192]
        b_all[:, (2 * k) * N:(2 * k + 1) * N] = cbT[:P]
        b_all[:, (2 * k + 1) * N:(2 * k + 2) * N] = cbT[P:]
    bx = np.zeros((P, N), dtype=np.float32)
    bx[0] = s1.astype(np.float32)
    bx[1] = s2.astype(np.float32)
    bx[2] = s3.astype(np.float32)
    bx[3:3 + H] = onehot.T * np.float32(BIGM)
    b_all[:, 6 * N:7 * N] = bx.astype(f16)

    # A side (per core): 7 chunks [128,128] per row tile, packed [128, RT*7*128]
    a_maps = []
    for core in range(N_CORES):
        a_all = np.zeros((P, RT * NCHUNK * P), dtype=f16)
        for rt in range(RT):
            r0 = core * RPC + rt * P
            rows = slice(r0, r0 + P)
            base = rt * NCHUNK * P
            for k, ca in enumerate(chunks_a):
                caT = np.ascontiguousarray(ca[rows].T)   # [256, 128]
                a_all[:, base + (2 * k) * P:base + (2 * k + 1) * P] = caT[:P]
                a_all[:, base + (2 * k + 1) * P:base + (2 * k + 2) * P] = caT[P:]
            ax = np.zeros((P, P), dtype=np.float32)
            ax[0:3, :] = 1.0
            ax[3:3 + H, :] = onehot[rows].T
            a_all[:, base + 6 * P:base + 7 * P] = ax.astype(f16)
        a_maps.append({"b_in": b_all, "a_in": a_all})
    return a_maps


def _run(in_maps, trace=False, **kw):
    global _compiled
    if _compiled is None:
        _compiled = _build()
    return run_bass_kernel_spmd(_compiled, in_maps, list(range(N_CORES)),
                                trace=trace, **kw)


def kernel(e_actv, e_ap, host):
    in_maps = _prep_inputs(e_actv, host)
    res = _run(in_maps)
    idx = np.concatenate([res.results[c]["out_idx"] for c in range(N_CORES)])
    e = np.asarray(e_actv)
    e_an = e[idx]
    return (np.asarray(e_actv), np.asarray(e_ap), e_an)


# revision 6
# speedup vs baseline: 1.3203x; 1.3203x over previous
"""Masked nearest-neighbor (AnchorTs2Vec e_an) Trainium2 kernel.

Problem: for e_actv [8192, 256] f32 and host ids [8192], compute
    d2[i,j] = |e_i|^2 + |e_j|^2 - 2 e_i.e_j   (masked BIG where host_i==host_j, incl. diag)
    idx[i]  = argmin_j d2[i,j]   (first index on ties, matching jnp.argmin)
    e_an    = e_actv[idx]
Returns (e_actv, e_ap, e_an) like the reference.

Distribution: rows sharded across 8 NeuronCores (1024 rows/core); the
column operand (all 8192 embeddings) is replicated to every core, so no
collective is needed.

Device computation per core (per 128-row tile):
  - One fused matmul chain computes the *masked* distance surrogate
        val[i,j] = sq_j - 2*G_ij + 32768*[host_i==host_j]
    directly in PSUM via an extended contraction dimension: an fp16
    hi/lo split of e_actv (G = Eh.Eh + Eh.El + El.Eh with exact
    power-of-two +-64 balancing of the cross terms to stay in fp16
    normal range; products are computed exactly by the PE, fp32
    accumulated => ~1e-5 abs error vs the ~1.8e-4 min argmin margin of
    this distance distribution), plus 3 fp16 splits of sq_j and a
    32768*onehot64(host) mask block. sq_i is omitted (constant per row,
    argmin-invariant); sqrt is omitted (monotone, argmin-invariant).
  - DVE prefix-min scan (tensor_tensor_scan) over the 16 PSUM column
    tiles, carry-chained -> running min r; gmin = r[:, -1].
  - ACT Sign(r - gmin) with accum_out: sum of [r_j > gmin] = exact
    first-occurrence argmin index (two half passes through a small fp8
    scratch). Cast int32, DMA out.

The B operand DRAM layout is column-tile-major ([ct][chunk][512]) so
its 16 DMAs pipeline with the matmul stream instead of serializing at
kernel start. Host (numpy) does only input staging (fp16 split,
transposes, one-hot) and the final trivial row gather e_actv[idx].
"""

import numpy as np

import concourse.tile as tile
from concourse import bacc, mybir
from concourse.bass_utils import run_bass_kernel_spmd

N, D, H = 8192, 256, 64
N_CORES = 8
RPC = N // N_CORES          # rows per core
P = 128                     # partitions
RT = RPC // P               # row tiles per core (8)
TS = 512                    # col tile size (one PSUM bank)
CT = N // TS                # col tiles (16)
NCHUNK = 7                  # A/B chunks: h0 h1 hs0 hs1 ls0 ls1 extras
CTW = NCHUNK * TS           # free-dim width of one B column-tile group
BIGM = 32768.0              # mask constant (fp16-safe, dominates all real d2)
SCALE = np.float32(64.0)    # exact power-of-two balancing for cross terms

f16 = np.float16

# chunk pairing for the extended contraction (a_chunk i pairs b_chunk i):
#   0,1: (-2Eh)^T    x Eh^T        -> -2 Eh.Eh
#   2,3: (-2Eh/64)^T x (El*64)^T   -> -2 Eh.El
#   4,5: (-2El*64)^T x (Eh/64)^T   -> -2 El.Eh
#   6:   extras: [1,1,1,onehot(host_i)] x [sq1,sq2,sq3, BIGM*onehot(host_j)]

_compiled = None


def _build():
    nc = bacc.Bacc("TRN2", target_bir_lowering=False, debug=False,
                   num_devices=N_CORES)
    b_in = nc.dram_tensor("b_in", [P, CT * CTW], mybir.dt.float16,
                          kind="ExternalInput").ap()
    a_in = nc.dram_tensor("a_in", [P, RT * NCHUNK * P], mybir.dt.float16,
                          kind="ExternalInput").ap()
    out_idx = nc.dram_tensor("out_idx", [RPC], mybir.dt.int32,
                             kind="ExternalOutput").ap()

    with tile.TileContext(nc) as tc:
        with tc.tile_pool(name="bp", bufs=1) as bp, \
             tc.tile_pool(name="apool", bufs=3) as apool, \
             tc.tile_pool(name="rp", bufs=2) as rp, \
             tc.tile_pool(name="sp", bufs=1) as sp, \
             tc.tile_pool(name="small", bufs=2) as small, \
             tc.tile_pool(name="psum", bufs=8, space="PSUM") as pp:
            # A tiles for rt0/rt1 first (small), then B column-tile groups in
            # consumption order; later A tiles prefetched inside the loop.
            def load_a(rt):
                at = apool.tile([P, NCHUNK * P], mybir.dt.float16, tag="a")
                nc.sync.dma_start(
                    at[:], a_in[:, rt * NCHUNK * P:(rt + 1) * NCHUNK * P])
                return at

            atiles = [load_a(0), load_a(1)]
            btile = bp.tile([P, CT * CTW], mybir.dt.float16, tag="b")
            for ct in range(CT):
                nc.sync.dma_start(btile[:, ct * CTW:(ct + 1) * CTW],
                                  b_in[:, ct * CTW:(ct + 1) * CTW])
            dummy = sp.tile([P, TS], mybir.dt.float32, tag="dummy")
            nc.vector.memset(dummy[:], 0.0)

            for rt in range(RT):
                at = atiles[rt]
                if rt + 2 < RT:
                    atiles.append(load_a(rt + 2))
                r = rp.tile([P, N], mybir.dt.float32, tag="r")
                for ct in range(CT):
                    ps = pp.tile([P, TS], mybir.dt.float32, tag="ps")
                    for c in range(NCHUNK):
                        lhsT = at[:, c * P:(c + 1) * P]
                        rhs = btile[:, ct * CTW + c * TS: ct * CTW + (c + 1) * TS]
                        nc.tensor.matmul(ps[:], lhsT, rhs,
                                         start=(c == 0), stop=(c == NCHUNK - 1))
                    init = BIGM * 2 if ct == 0 else r[:, ct * TS - 1: ct * TS]
                    nc.vector.tensor_tensor_scan(
                        r[:, ct * TS:(ct + 1) * TS], ps[:], dummy[:],
                        initial=init,
                        op0=mybir.AluOpType.min, op1=mybir.AluOpType.bypass)

                negmin = small.tile([P, 1], mybir.dt.float32, tag="negmin")
                nc.vector.tensor_scalar_mul(negmin[:], r[:, N - 1:N], -1.0)
                NQ = 4
                sgn = sp.tile([P, N // NQ], mybir.dt.float8e4, tag="sgn")
                cnts = small.tile([P, NQ], mybir.dt.float32, tag="cnts")
                for k in range(NQ):
                    nc.scalar.activation(sgn[:], r[:, k * (N // NQ):(k + 1) * (N // NQ)],
                                         mybir.ActivationFunctionType.Sign,
                                         bias=negmin[:, 0:1], scale=1.0,
                                         accum_out=cnts[:, k:k + 1])
                cnt = small.tile([P, 1], mybir.dt.float32, tag="cnt")
                nc.vector.reduce_sum(cnt[:], cnts[:], axis=mybir.AxisListType.X)
                cnt_i = small.tile([P, 1], mybir.dt.int32, tag="cnti")
                nc.vector.tensor_copy(cnt_i[:], cnt[:])
                nc.sync.dma_start(out_idx[rt * P:(rt + 1) * P], cnt_i[:, 0])

    nc.compile()
    return nc


def _prep_inputs(e_actv: np.ndarray, host: np.ndarray):
    e = np.ascontiguousarray(np.asarray(e_actv, dtype=np.float32))
    hostv = np.asarray(host).astype(np.int64)

    eh = e.astype(f16)
    ehf = eh.astype(np.float32)
    elf = e - ehf

    chunks_b = [
        eh,                           # b0/b1: Eh
        (elf * SCALE).astype(f16),    # b2/b3: El*64
        (ehf / SCALE).astype(f16),    # b4/b5: Eh/64
    ]
    chunks_a = [
        (-2.0 * ehf).astype(f16),          # a0/a1: -2Eh
        (-2.0 * ehf / SCALE).astype(f16),  # a2/a3: -2Eh/64
        (-2.0 * elf * SCALE).astype(f16),  # a4/a5: -2El*64
    ]

    sq = (e.astype(np.float64) * e.astype(np.float64)).sum(1)
    s1 = sq.astype(np.float32).astype(f16)
    r1 = (sq - s1.astype(np.float64)).astype(np.float32)
    s2 = r1.astype(f16)
    s3 = (r1 - s2.astype(np.float32)).astype(f16)

    onehot = np.zeros((N, H), dtype=np.float32)
    onehot[np.arange(N), hostv] = 1.0

    # B chunk stack [7, 128, 8192] then repack column-tile-major
    bstack = np.zeros((NCHUNK, P, N), dtype=f16)
    for k, cb in enumerate(chunks_b):
        cbT = np.ascontiguousarray(cb.T)       # [256, 8192]
        bstack[2 * k] = cbT[:P]
        bstack[2 * k + 1] = cbT[P:]
    bx = np.zeros((P, N), dtype=np.float32)
    bx[0] = s1.astype(np.float32)
    bx[1] = s2.astype(np.float32)
    bx[2] = s3.astype(np.float32)
    bx[3:3 + H] = onehot.T * np.float32(BIGM)
    bstack[6] = bx.astype(f16)
    # [chunk, P, ct, 512] -> [P, ct, chunk, 512]
    b_all = np.ascontiguousarray(
        bstack.reshape(NCHUNK, P, CT, TS).transpose(1, 2, 0, 3)
    ).reshape(P, CT * CTW)

    # A side (per core): 7 chunks [128,128] per row tile, packed [128, RT*7*128]
    a_maps = []
    for core in range(N_CORES):
        a_all = np.zeros((P, RT * NCHUNK * P), dtype=f16)
        for rt in range(RT):
            r0 = core * RPC + rt * P
            rows = slice(r0, r0 + P)
            base = rt * NCHUNK * P
            for k, ca in enumerate(chunks_a):
                caT = np.ascontiguousarray(ca[rows].T)   # [256, 128]
                a_all[:, base + (2 * k) * P:base + (2 * k + 1) * P] = caT[:P]
                a_all[:, base + (2 * k + 1) * P:base + (2 * k + 2) * P] = caT[P:]
            ax = np.zeros((P, P), dtype=np.float32)
            ax[0:3, :] = 1.0
            ax[3:3 + H, :] = onehot[rows].T
            a_all[:, base + 6 * P:base + 7 * P] = ax.astype(f16)
        a_maps.append({"b_in": b_all, "a_in": a_all})
    return a_maps


def _run(in_maps, trace=False, **kw):
    global _compiled
    if _compiled is None:
        _compiled = _build()
    return run_bass_kernel_spmd(_compiled, in_maps, list(range(N_CORES)),
                                trace=trace, **kw)


def kernel(e_actv, e_ap, host):
    in_maps = _prep_inputs(e_actv, host)
    res = _run(in_maps)
    idx = np.concatenate([res.results[c]["out_idx"] for c in range(N_CORES)])
    e = np.asarray(e_actv)
    e_an = e[idx]
    return (np.asarray(e_actv), np.asarray(e_ap), e_an)


# revision 9
# speedup vs baseline: 1.3635x; 1.0328x over previous
"""Masked nearest-neighbor (AnchorTs2Vec e_an) Trainium2 kernel.

Problem: for e_actv [8192, 256] f32 and host ids [8192], compute
    d2[i,j] = |e_i|^2 + |e_j|^2 - 2 e_i.e_j   (masked BIG where host_i==host_j, incl. diag)
    idx[i]  = argmin_j d2[i,j]   (first index on ties, matching jnp.argmin)
    e_an    = e_actv[idx]
Returns (e_actv, e_ap, e_an) like the reference.

Distribution: rows sharded across 8 NeuronCores (1024 rows/core); the
column operand (all 8192 embeddings) is replicated to every core, so no
collective is needed.

Device computation per core (per 128-row tile):
  - One fused matmul chain computes the *masked* distance surrogate
        val[i,j] = sq_j - 2*G_ij + 32768*[host_i==host_j]
    directly in PSUM via an extended contraction dimension: an fp16
    hi/lo split of e_actv (G = Eh.Eh + Eh.El + El.Eh with exact
    power-of-two +-64 balancing of the cross terms to stay in fp16
    normal range; products are computed exactly by the PE, fp32
    accumulated => ~1e-5 abs error vs the ~1.8e-4 min argmin margin of
    this distance distribution), plus 3 fp16 splits of sq_j and a
    32768*onehot64(host) mask block. sq_i is omitted (constant per row,
    argmin-invariant); sqrt is omitted (monotone, argmin-invariant).
  - DVE prefix-min scan (tensor_tensor_scan) over the 16 PSUM column
    tiles, carry-chained -> running min r; gmin = r[:, -1].
  - ACT Sign(r - gmin) with accum_out: sum of [r_j > gmin] = exact
    first-occurrence argmin index (two half passes through a small fp8
    scratch). Cast int32, DMA out.

The B operand DRAM layout is column-tile-major ([ct][chunk][512]) so
its 16 DMAs pipeline with the matmul stream instead of serializing at
kernel start. Host (numpy) does only input staging (fp16 split,
transposes, one-hot) and the final trivial row gather e_actv[idx].
"""

import numpy as np

import concourse.tile as tile
from concourse import bacc, mybir
from concourse.bass_utils import run_bass_kernel_spmd

N, D, H = 8192, 256, 64
N_CORES = 8
RPC = N // N_CORES          # rows per core
P = 128                     # partitions
RT = RPC // P               # row tiles per core (8)
TS = 512                    # col tile size (one PSUM bank)
CT = N // TS                # col tiles (16)
NCHUNK = 7                  # A/B chunks: h0 h1 hs0 hs1 ls0 ls1 extras
CTW = NCHUNK * TS           # free-dim width of one B column-tile group
BIGM = 32768.0              # mask constant (fp16-safe, dominates all real d2)
SCALE = np.float32(64.0)    # exact power-of-two balancing for cross terms

f16 = np.float16

# chunk pairing for the extended contraction (a_chunk i pairs b_chunk i):
#   0,1: (-2Eh)^T    x Eh^T        -> -2 Eh.Eh
#   2,3: (-2Eh/64)^T x (El*64)^T   -> -2 Eh.El
#   4,5: (-2El*64)^T x (Eh/64)^T   -> -2 El.Eh
#   6:   extras: [1,1,1,onehot(host_i)] x [sq1,sq2,sq3, BIGM*onehot(host_j)]

_compiled = None


def _build():
    nc = bacc.Bacc("TRN2", target_bir_lowering=False, debug=False,
                   num_devices=N_CORES)
    # contiguous per-column-tile-group / per-row-tile DRAM layouts
    b_in = nc.dram_tensor("b_in", [CT, P, CTW], mybir.dt.float16,
                          kind="ExternalInput").ap()
    a_in = nc.dram_tensor("a_in", [RT, P, NCHUNK * P], mybir.dt.float16,
                          kind="ExternalInput").ap()
    out_idx = nc.dram_tensor("out_idx", [RPC], mybir.dt.int32,
                             kind="ExternalOutput").ap()

    with tile.TileContext(nc) as tc:
        with tc.tile_pool(name="bp", bufs=1) as bp, \
             tc.tile_pool(name="apool", bufs=4) as apool, \
             tc.tile_pool(name="rp", bufs=2) as rp, \
             tc.tile_pool(name="sp", bufs=1) as sp, \
             tc.tile_pool(name="small", bufs=2) as small, \
             tc.tile_pool(name="psum", bufs=8, space="PSUM") as pp:
            def load_a(rt):
                at = apool.tile([P, NCHUNK * P], mybir.dt.float16, tag="a")
                nc.sync.dma_start(at[:], a_in[rt])
                return at

            # A for the first row-tile pair, then B groups in consumption order
            atiles = [load_a(0), load_a(1)]
            btile = bp.tile([P, CT * CTW], mybir.dt.float16, tag="b")
            for ct in range(CT):
                nc.sync.dma_start(btile[:, ct * CTW:(ct + 1) * CTW], b_in[ct])
            dummy = sp.tile([P, TS], mybir.dt.float32, tag="dummy")
            nc.vector.memset(dummy[:], 0.0)

            # Row-tile PAIRS, column-major inside a pair: during the initial
            # B-group DMA chase the PE has 2 row-tiles of work per arriving
            # group, so it never starves; scans chase per-tile.
            for pair in range(RT // 2):
                rts = (2 * pair, 2 * pair + 1)
                ats = (atiles[rts[0]], atiles[rts[1]])
                if pair + 1 < RT // 2:
                    atiles.append(load_a(2 * pair + 2))
                    atiles.append(load_a(2 * pair + 3))
                rpair = []
                for _ri in range(2):
                    r_t = rp.tile([P, N], mybir.dt.float32, tag="r")
                    rpair.append(r_t)
                for ct in range(CT):
                    for k in range(2):
                        ps = pp.tile([P, TS], mybir.dt.float32, tag="ps")
                        for c in range(NCHUNK):
                            lhsT = ats[k][:, c * P:(c + 1) * P]
                            rhs = btile[:, ct * CTW + c * TS: ct * CTW + (c + 1) * TS]
                            nc.tensor.matmul(ps[:], lhsT, rhs,
                                             start=(c == 0),
                                             stop=(c == NCHUNK - 1))
                        r = rpair[k]
                        init = BIGM * 2 if ct == 0 else r[:, ct * TS - 1: ct * TS]
                        nc.vector.tensor_tensor_scan(
                            r[:, ct * TS:(ct + 1) * TS], ps[:], dummy[:],
                            initial=init,
                            op0=mybir.AluOpType.min, op1=mybir.AluOpType.bypass)

                NQ = 4
                for k in range(2):
                    r = rpair[k]
                    negmin = small.tile([P, 1], mybir.dt.float32, tag="negmin")
                    nc.vector.tensor_scalar_mul(negmin[:], r[:, N - 1:N], -1.0)
                    sgn = sp.tile([P, N // NQ], mybir.dt.float8e4, tag="sgn")
                    cnts = small.tile([P, NQ], mybir.dt.float32, tag="cnts")
                    for q in range(NQ):
                        nc.scalar.activation(
                            sgn[:], r[:, q * (N // NQ):(q + 1) * (N // NQ)],
                            mybir.ActivationFunctionType.Sign,
                            bias=negmin[:, 0:1], scale=1.0,
                            accum_out=cnts[:, q:q + 1])
                    cnt = small.tile([P, 1], mybir.dt.float32, tag="cnt")
                    nc.vector.reduce_sum(cnt[:], cnts[:], axis=mybir.AxisListType.X)
                    cnt_i = small.tile([P, 1], mybir.dt.int32, tag="cnti")
                    nc.vector.tensor_copy(cnt_i[:], cnt[:])
                    nc.sync.dma_start(out_idx[rts[k] * P:(rts[k] + 1) * P],
                                      cnt_i[:, 0])

    nc.compile()
    return nc


def _prep_inputs(e_actv: np.ndarray, host: np.ndarray):
    e = np.ascontiguousarray(np.asarray(e_actv, dtype=np.float32))
    hostv = np.asarray(host).astype(np.int64)

    eh = e.astype(f16)
    ehf = eh.astype(np.float32)
    elf = e - ehf

    chunks_b = [
        eh,                           # b0/b1: Eh
        (elf * SCALE).astype(f16),    # b2/b3: El*64
        (ehf / SCALE).astype(f16),    # b4/b5: Eh/64
    ]
    chunks_a = [
        (-2.0 * ehf).astype(f16),          # a0/a1: -2Eh
        (-2.0 * ehf / SCALE).astype(f16),  # a2/a3: -2Eh/64
        (-2.0 * elf * SCALE).astype(f16),  # a4/a5: -2El*64
    ]

    sq = (e.astype(np.float64) * e.astype(np.float64)).sum(1)
    s1 = sq.astype(np.float32).astype(f16)
    r1 = (sq - s1.astype(np.float64)).astype(np.float32)
    s2 = r1.astype(f16)
    s3 = (r1 - s2.astype(np.float32)).astype(f16)

    onehot = np.zeros((N, H), dtype=np.float32)
    onehot[np.arange(N), hostv] = 1.0

    # B chunk stack [7, 128, 8192] then repack column-tile-major
    bstack = np.zeros((NCHUNK, P, N), dtype=f16)
    for k, cb in enumerate(chunks_b):
        cbT = np.ascontiguousarray(cb.T)       # [256, 8192]
        bstack[2 * k] = cbT[:P]
        bstack[2 * k + 1] = cbT[P:]
    bx = np.zeros((P, N), dtype=np.float32)
    bx[0] = s1.astype(np.float32)
    bx[1] = s2.astype(np.float32)
    bx[2] = s3.astype(np.float32)
    bx[3:3 + H] = onehot.T * np.float32(BIGM)
    bstack[6] = bx.astype(f16)
    # [chunk, P, ct, 512] -> [ct, P, chunk, 512]  (contiguous per ct group)
    b_all = np.ascontiguousarray(
        bstack.reshape(NCHUNK, P, CT, TS).transpose(2, 1, 0, 3)
    ).reshape(CT, P, CTW)

    # A side (per core): 7 chunks [128,128] per row tile, [RT, 128, 7*128]
    a_maps = []
    for core in range(N_CORES):
        a_all = np.zeros((RT, P, NCHUNK * P), dtype=f16)
        for rt in range(RT):
            r0 = core * RPC + rt * P
            rows = slice(r0, r0 + P)
            for k, ca in enumerate(chunks_a):
                caT = np.ascontiguousarray(ca[rows].T)   # [256, 128]
                a_all[rt, :, (2 * k) * P:(2 * k + 1) * P] = caT[:P]
                a_all[rt, :, (2 * k + 1) * P:(2 * k + 2) * P] = caT[P:]
            ax = np.zeros((P, P), dtype=np.float32)
            ax[0:3, :] = 1.0
            ax[3:3 + H, :] = onehot[rows].T
            a_all[rt, :, 6 * P:7 * P] = ax.astype(f16)
        a_maps.append({"b_in": b_all, "a_in": a_all})
    return a_maps


def _run(in_maps, trace=False, **kw):
    global _compiled
    if _compiled is None:
        _compiled = _build()
    return run_bass_kernel_spmd(_compiled, in_maps, list(range(N_CORES)),
                                trace=trace, **kw)


def kernel(e_actv, e_ap, host):
    in_maps = _prep_inputs(e_actv, host)
    res = _run(in_maps)
    idx = np.concatenate([res.results[c]["out_idx"] for c in range(N_CORES)])
    e = np.asarray(e_actv)
    e_an = e[idx]
    return (np.asarray(e_actv), np.asarray(e_ap), e_an)


# revision 10
# speedup vs baseline: 1.4004x; 1.0270x over previous
"""Masked nearest-neighbor (AnchorTs2Vec e_an) Trainium2 kernel.

Problem: for e_actv [8192, 256] f32 and host ids [8192], compute
    d2[i,j] = |e_i|^2 + |e_j|^2 - 2 e_i.e_j   (masked BIG where host_i==host_j, incl. diag)
    idx[i]  = argmin_j d2[i,j]   (first index on ties, matching jnp.argmin)
    e_an    = e_actv[idx]
Returns (e_actv, e_ap, e_an) like the reference.

Distribution: rows sharded across 8 NeuronCores (1024 rows/core); the
column operand (all 8192 embeddings) is replicated to every core, so no
collective is needed.

Device computation per core (per 128-row tile):
  - One fused matmul chain computes a NEGATED masked distance surrogate
        nval[i,j] = 2*G_ij - sq_j - 32768*[host_i==host_j]
    in PSUM via an extended contraction dimension: e_actv in fp16
    (products computed exactly by the PE, fp32 accumulated; total error
    vs the exact distance surrogate is bounded by ~0.06), plus 3 fp16
    splits of sq_j and a -32768*onehot64(host) mask block. sq_i omitted
    (row constant, argmin-invariant); sqrt omitted (monotone).
  - ACT copies PSUM->SBUF (frees PSUM banks for the next row tile).
  - DVE max8 -> top-8 values: top1 = -min distance surrogate; top2
    gives the runner-up for ambiguity detection.
  - DVE is_equal(val, top1) -> uint8 one-hot mask, DMA'd to the host.
Host side: idx = mask.argmax(1) (exact first-index semantics); rows
where top1-top2 < RESCUE_THR (fp16 precision cannot certify the true
argmin) are recomputed exactly in fp32 numpy (~100-200 of 8192 rows);
final gather e_actv[idx].
"""

import numpy as np

import concourse.tile as tile
from concourse import bacc, mybir
from concourse.bass_utils import run_bass_kernel_spmd

N, D, H = 8192, 256, 64
N_CORES = 8
RPC = N // N_CORES          # rows per core
P = 128                     # partitions
RT = RPC // P               # row tiles per core (8)
TS = 512                    # matmul free-dim (one PSUM bank)
GW = 2048                   # column group width (4 PSUM banks)
NG = N // GW                # column groups (4)
NCHUNK = 3                  # contraction chunks: Eh0, Eh1, extras
CTW = NCHUNK * GW           # B bytes-free per column group (in elements)
BIGM = 32768.0
RESCUE_THR = 0.30           # rescue when top1-top2 below this (fp16 err ~0.06)

f16 = np.float16

_compiled = None


def _build():
    nc = bacc.Bacc("TRN2", target_bir_lowering=False, debug=False,
                   num_devices=N_CORES)
    b_in = nc.dram_tensor("b_in", [NG, P, CTW], mybir.dt.float16,
                          kind="ExternalInput").ap()
    a_in = nc.dram_tensor("a_in", [RT, P, NCHUNK * P], mybir.dt.float16,
                          kind="ExternalInput").ap()
    out_mask = nc.dram_tensor("out_mask", [RPC, N], mybir.dt.uint8,
                              kind="ExternalOutput").ap()
    out_top = nc.dram_tensor("out_top", [RPC, 2], mybir.dt.float32,
                             kind="ExternalOutput").ap()

    with tile.TileContext(nc) as tc:
        with tc.tile_pool(name="bp", bufs=1) as bp, \
             tc.tile_pool(name="apool", bufs=4) as apool, \
             tc.tile_pool(name="vp", bufs=2) as vp, \
             tc.tile_pool(name="mp", bufs=2) as mp, \
             tc.tile_pool(name="small", bufs=2) as small, \
             tc.tile_pool(name="psum", bufs=2, space="PSUM") as pp:
            def load_a(rt):
                at = apool.tile([P, NCHUNK * P], mybir.dt.float16, tag="a")
                nc.sync.dma_start(at[:], a_in[rt])
                return at

            atiles = [load_a(0), load_a(1)]
            btile = bp.tile([P, NG * CTW], mybir.dt.float16, tag="b")
            for g in range(NG):
                nc.sync.dma_start(btile[:, g * CTW:(g + 1) * CTW], b_in[g])

            # row-tile pairs, column-group-major within a pair (DMA chase)
            for pair in range(RT // 2):
                rts = (2 * pair, 2 * pair + 1)
                ats = (atiles[rts[0]], atiles[rts[1]])
                if pair + 1 < RT // 2:
                    atiles.append(load_a(2 * pair + 2))
                    atiles.append(load_a(2 * pair + 3))
                vpair = []
                for _vi in range(2):
                    v_t = vp.tile([P, N], mybir.dt.float32, tag="val")
                    vpair.append(v_t)
                for g in range(NG):
                    for k in range(2):
                        ps = pp.tile([P, GW], mybir.dt.float32, tag="ps")
                        for s in range(GW // TS):       # 4 col subtiles
                            for c in range(NCHUNK):
                                lhsT = ats[k][:, c * P:(c + 1) * P]
                                boff = g * CTW + c * GW + s * TS
                                rhs = btile[:, boff:boff + TS]
                                nc.tensor.matmul(
                                    ps[:, s * TS:(s + 1) * TS], lhsT, rhs,
                                    start=(c == 0), stop=(c == NCHUNK - 1))
                        # free the 4 banks promptly
                        nc.scalar.copy(vpair[k][:, g * GW:(g + 1) * GW], ps[:])

                for k in range(2):
                    v = vpair[k]
                    t8 = small.tile([P, 8], mybir.dt.float32, tag="t8")
                    nc.vector.max(out=t8[:], in_=v[:])
                    mask = mp.tile([P, N], mybir.dt.uint8, tag="mask")
                    nc.vector.tensor_scalar(mask[:], v[:], t8[:, 0:1], None,
                                            op0=mybir.AluOpType.is_equal)
                    r0 = rts[k] * P
                    nc.sync.dma_start(out_mask[r0:r0 + P, :], mask[:])
                    nc.sync.dma_start(out_top[r0:r0 + P, :], t8[:, 0:2])

    nc.compile()
    return nc


def _prep_inputs(e_actv: np.ndarray, host: np.ndarray):
    e = np.ascontiguousarray(np.asarray(e_actv, dtype=np.float32))
    hostv = np.asarray(host).astype(np.int64)

    eh = e.astype(f16)                       # fp16 embeddings (B side)
    a1 = (2.0 * eh.astype(np.float32)).astype(f16)   # +2*Eh (A side)

    sq = (e.astype(np.float64) * e.astype(np.float64)).sum(1)
    s1 = sq.astype(np.float32).astype(f16)
    r1 = (sq - s1.astype(np.float64)).astype(np.float32)
    s2 = r1.astype(f16)
    s3 = (r1 - s2.astype(np.float32)).astype(f16)

    onehot = np.zeros((N, H), dtype=np.float32)
    onehot[np.arange(N), hostv] = 1.0

    # B chunk stack [3, 128, 8192]: Eh^T halves + negated extras
    bstack = np.zeros((NCHUNK, P, N), dtype=f16)
    ehT = np.ascontiguousarray(eh.T)
    bstack[0] = ehT[:P]
    bstack[1] = ehT[P:]
    bx = np.zeros((P, N), dtype=np.float32)
    bx[0] = -s1.astype(np.float32)
    bx[1] = -s2.astype(np.float32)
    bx[2] = -s3.astype(np.float32)
    bx[3:3 + H] = onehot.T * np.float32(-BIGM)
    bstack[2] = bx.astype(f16)
    # -> [group, P, chunk, 2048] contiguous per group
    b_all = np.ascontiguousarray(
        bstack.reshape(NCHUNK, P, NG, GW).transpose(2, 1, 0, 3)
    ).reshape(NG, P, CTW)

    a_maps = []
    for core in range(N_CORES):
        a_all = np.zeros((RT, P, NCHUNK * P), dtype=f16)
        for rt in range(RT):
            r0 = core * RPC + rt * P
            rows = slice(r0, r0 + P)
            a1T = np.ascontiguousarray(a1[rows].T)       # [256, 128]
            a_all[rt, :, 0:P] = a1T[:P]
            a_all[rt, :, P:2 * P] = a1T[P:]
            ax = np.zeros((P, P), dtype=np.float32)
            ax[0:3, :] = 1.0
            ax[3:3 + H, :] = onehot[rows].T
            a_all[rt, :, 2 * P:3 * P] = ax.astype(f16)
        a_maps.append({"b_in": b_all, "a_in": a_all})
    return a_maps


def _run(in_maps, trace=False, **kw):
    global _compiled
    if _compiled is None:
        _compiled = _build()
    return run_bass_kernel_spmd(_compiled, in_maps, list(range(N_CORES)),
                                trace=trace, **kw)


def _exact_rows(e, hostv, rows):
    """Exact fp32 masked argmin for the given rows (reference semantics)."""
    sq = (e * e).sum(1)
    G = e[rows] @ e.T
    d2 = sq[rows][:, None] + sq[None, :] - 2.0 * G
    d2 = np.where(hostv[rows][:, None] == hostv[None, :], np.float32(1e30), d2)
    return d2.argmin(1)


def kernel(e_actv, e_ap, host):
    e = np.ascontiguousarray(np.asarray(e_actv, dtype=np.float32))
    hostv = np.asarray(host).astype(np.int64)
    in_maps = _prep_inputs(e, hostv)
    res = _run(in_maps)

    masks = np.concatenate([res.results[c]["out_mask"] for c in range(N_CORES)])
    tops = np.concatenate([res.results[c]["out_top"] for c in range(N_CORES)])
    idx = masks.argmax(axis=1)

    margin = tops[:, 0] - tops[:, 1]
    rescue = np.where((margin < RESCUE_THR) | (masks.max(axis=1) == 0))[0]
    if len(rescue):
        idx[rescue] = _exact_rows(e, hostv, rescue)

    e_an = np.asarray(e_actv)[idx]
    return (np.asarray(e_actv), np.asarray(e_ap), e_an)


# revision 11
# speedup vs baseline: 1.5512x; 1.1077x over previous
"""Masked nearest-neighbor (AnchorTs2Vec e_an) Trainium2 kernel.

Problem: for e_actv [8192, 256] f32 and host ids [8192], compute
    d2[i,j] = |e_i|^2 + |e_j|^2 - 2 e_i.e_j   (masked BIG where host_i==host_j, incl. diag)
    idx[i]  = argmin_j d2[i,j]   (first index on ties, matching jnp.argmin)
    e_an    = e_actv[idx]
Returns (e_actv, e_ap, e_an) like the reference.

Distribution: rows sharded across 8 NeuronCores (1024 rows/core); the
column operand (all 8192 embeddings) is replicated to every core, so no
collective is needed.

Device computation per core (per 128-row tile):
  - One fused matmul chain computes a NEGATED masked distance surrogate
        nval[i,j] = 2*G_ij - sq_j - 32768*[host_i==host_j]
    in PSUM via an extended contraction: fp16 hi split Eh=fp16(e) plus
    the Eh.El cross term (El = e-Eh, exact power-of-two +-64 balancing),
    G ~ Eh.Eh + Eh.El, |error| <~ 4e-3; plus 3 fp16 splits of sq_j and
    a -32768*onehot64(host) mask block (5 K-chunks total -- sized so
    the warm PE stream ~ matches the DVE span, keeping the PE dense and
    HAM-warm). sq_i omitted (row constant), sqrt omitted (monotone).
  - ACT copies PSUM -> SBUF as fp16 (frees PSUM banks).
  - DVE max8 per column group + combine: top1 = -min, top2 = runner-up.
  - DVE is_equal(val, top1) -> uint8 one-hot mask, DMA'd to the host.
Host: idx = mask.argmax(1) (exact first-index); rows whose device
margin (top1-top2) cannot certify the true argmin (fp16 copy quant
0.25 + model err) are recomputed exactly in fp32 numpy (<~100 rows);
final gather e_actv[idx].
"""

import numpy as np

import concourse.tile as tile
from concourse import bacc, mybir
from concourse.bass_utils import run_bass_kernel_spmd

N, D, H = 8192, 256, 64
N_CORES = 8
RPC = N // N_CORES          # rows per core
P = 128                     # partitions
RT = RPC // P               # row tiles per core (8)
TS = 512                    # matmul free-dim (one PSUM bank)
GW = 2048                   # column group width (4 PSUM banks)
NG = N // GW                # column groups (4)
NCHUNK = 5                  # Eh0 Eh1 EhS0 EhS1 extras
CTW = NCHUNK * GW
BIGM = 32768.0
SCALE = np.float32(64.0)
RESCUE_THR = 1.0            # device margin below which the host recomputes

f16 = np.float16

_compiled = None


def _build():
    nc = bacc.Bacc("TRN2", target_bir_lowering=False, debug=False,
                   num_devices=N_CORES)
    b_in = nc.dram_tensor("b_in", [NG, P, CTW], mybir.dt.float16,
                          kind="ExternalInput").ap()
    a_in = nc.dram_tensor("a_in", [RT, P, NCHUNK * P], mybir.dt.float16,
                          kind="ExternalInput").ap()
    out_mask = nc.dram_tensor("out_mask", [RPC, N], mybir.dt.uint8,
                              kind="ExternalOutput").ap()
    out_top = nc.dram_tensor("out_top", [RPC, 2], mybir.dt.float32,
                             kind="ExternalOutput").ap()

    with tile.TileContext(nc) as tc:
        with tc.tile_pool(name="bp", bufs=1) as bp, \
             tc.tile_pool(name="apool", bufs=4) as apool, \
             tc.tile_pool(name="vp", bufs=4) as vp, \
             tc.tile_pool(name="mp", bufs=2) as mp, \
             tc.tile_pool(name="small", bufs=4) as small, \
             tc.tile_pool(name="psum", bufs=2, space="PSUM") as pp:
            def load_a(rt):
                at = apool.tile([P, NCHUNK * P], mybir.dt.float16, tag="a")
                nc.sync.dma_start(at[:], a_in[rt])
                return at

            atiles = [load_a(0), load_a(1)]
            btile = bp.tile([P, NG * CTW], mybir.dt.float16, tag="b")
            for g in range(NG):
                nc.sync.dma_start(btile[:, g * CTW:(g + 1) * CTW], b_in[g])

            for pair in range(RT // 2):
                rts = (2 * pair, 2 * pair + 1)
                ats = (atiles[rts[0]], atiles[rts[1]])
                if pair + 1 < RT // 2:
                    atiles.append(load_a(2 * pair + 2))
                    atiles.append(load_a(2 * pair + 3))
                vpair, t8s = [], []
                for _vi in range(2):
                    v_t = vp.tile([P, N], mybir.dt.float16, tag="val")
                    vpair.append(v_t)
                    t8_t = small.tile([P, 40], mybir.dt.float32, tag="t8")
                    t8s.append(t8_t)
                for g in range(NG):
                    for k in range(2):
                        ps = pp.tile([P, GW], mybir.dt.float32, tag="ps")
                        for s in range(GW // TS):
                            for c in range(NCHUNK):
                                lhsT = ats[k][:, c * P:(c + 1) * P]
                                boff = g * CTW + c * GW + s * TS
                                rhs = btile[:, boff:boff + TS]
                                nc.tensor.matmul(
                                    ps[:, s * TS:(s + 1) * TS], lhsT, rhs,
                                    start=(c == 0), stop=(c == NCHUNK - 1))
                        nc.scalar.activation(vpair[k][:, g * GW:(g + 1) * GW],
                                             ps[:],
                                             mybir.ActivationFunctionType.Copy)
                        # per-group top8 (spreads DVE work, shortens tail)
                        nc.vector.max(out=t8s[k][:, 8 + g * 8:16 + g * 8],
                                      in_=vpair[k][:, g * GW:(g + 1) * GW])

                for k in range(2):
                    t8 = t8s[k]
                    nc.vector.max(out=t8[:, 0:8], in_=t8[:, 8:40])
                    mask = mp.tile([P, N], mybir.dt.uint8, tag="mask")
                    nc.vector.tensor_scalar(mask[:], vpair[k][:], t8[:, 0:1],
                                            None,
                                            op0=mybir.AluOpType.is_equal)
                    r0 = rts[k] * P
                    nc.sync.dma_start(out_mask[r0:r0 + P, :], mask[:])
                    nc.sync.dma_start(out_top[r0:r0 + P, :], t8[:, 0:2])

    nc.compile()
    return nc


def _prep_inputs(e_actv: np.ndarray, host: np.ndarray):
    e = np.ascontiguousarray(np.asarray(e_actv, dtype=np.float32))
    hostv = np.asarray(host).astype(np.int64)

    eh = e.astype(f16)
    ehf = eh.astype(np.float32)
    elf = e - ehf

    chunks_b = [eh, (elf * SCALE).astype(f16)]
    chunks_a = [(2.0 * ehf).astype(f16), (2.0 * ehf / SCALE).astype(f16)]

    sq = (e.astype(np.float64) * e.astype(np.float64)).sum(1)
    s1 = sq.astype(np.float32).astype(f16)
    r1 = (sq - s1.astype(np.float64)).astype(np.float32)
    s2 = r1.astype(f16)
    s3 = (r1 - s2.astype(np.float32)).astype(f16)

    onehot = np.zeros((N, H), dtype=np.float32)
    onehot[np.arange(N), hostv] = 1.0

    bstack = np.zeros((NCHUNK, P, N), dtype=f16)
    for kk, cb in enumerate(chunks_b):
        cbT = np.ascontiguousarray(cb.T)
        bstack[2 * kk] = cbT[:P]
        bstack[2 * kk + 1] = cbT[P:]
    bx = np.zeros((P, N), dtype=np.float32)
    bx[0] = -s1.astype(np.float32)
    bx[1] = -s2.astype(np.float32)
    bx[2] = -s3.astype(np.float32)
    bx[3:3 + H] = onehot.T * np.float32(-BIGM)
    bstack[4] = bx.astype(f16)
    b_all = np.ascontiguousarray(
        bstack.reshape(NCHUNK, P, NG, GW).transpose(2, 1, 0, 3)
    ).reshape(NG, P, CTW)

    a_maps = []
    for core in range(N_CORES):
        a_all = np.zeros((RT, P, NCHUNK * P), dtype=f16)
        for rt in range(RT):
            r0 = core * RPC + rt * P
            rows = slice(r0, r0 + P)
            for kk, ca in enumerate(chunks_a):
                caT = np.ascontiguousarray(ca[rows].T)
                a_all[rt, :, (2 * kk) * P:(2 * kk + 1) * P] = caT[:P]
                a_all[rt, :, (2 * kk + 1) * P:(2 * kk + 2) * P] = caT[P:]
            ax = np.zeros((P, P), dtype=np.float32)
            ax[0:3, :] = 1.0
            ax[3:3 + H, :] = onehot[rows].T
            a_all[rt, :, 4 * P:5 * P] = ax.astype(f16)
        a_maps.append({"b_in": b_all, "a_in": a_all})
    return a_maps


def _run(in_maps, trace=False, **kw):
    global _compiled
    if _compiled is None:
        _compiled = _build()
    return run_bass_kernel_spmd(_compiled, in_maps, list(range(N_CORES)),
                                trace=trace, **kw)


def _exact_rows(e, hostv, rows):
    """Exact fp32 masked argmin for the given rows (reference semantics)."""
    sq = (e * e).sum(1)
    G = e[rows] @ e.T
    d2 = sq[rows][:, None] + sq[None, :] - 2.0 * G
    d2 = np.where(hostv[rows][:, None] == hostv[None, :], np.float32(1e30), d2)
    return d2.argmin(1)


def kernel(e_actv, e_ap, host):
    e = np.ascontiguousarray(np.asarray(e_actv, dtype=np.float32))
    hostv = np.asarray(host).astype(np.int64)
    in_maps = _prep_inputs(e, hostv)
    res = _run(in_maps)

    masks = np.concatenate([res.results[c]["out_mask"] for c in range(N_CORES)])
    tops = np.concatenate([res.results[c]["out_top"] for c in range(N_CORES)])
    idx = masks.argmax(axis=1)

    margin = tops[:, 0] - tops[:, 1]
    rescue = np.where((margin < RESCUE_THR) | (masks.max(axis=1) == 0))[0]
    if len(rescue):
        idx[rescue] = _exact_rows(e, hostv, rescue)

    e_an = np.asarray(e_actv)[idx]
    return (np.asarray(e_actv), np.asarray(e_ap), e_an)


# revision 12
# speedup vs baseline: 1.9825x; 1.2780x over previous
"""Masked nearest-neighbor (AnchorTs2Vec e_an) Trainium2 kernel.

Problem: for e_actv [8192, 256] f32 and host ids [8192], compute
    d2[i,j] = |e_i|^2 + |e_j|^2 - 2 e_i.e_j   (masked BIG where host_i==host_j, incl. diag)
    idx[i]  = argmin_j d2[i,j]   (first index on ties, matching jnp.argmin)
    e_an    = e_actv[idx]
Returns (e_actv, e_ap, e_an) like the reference.

Distribution: rows sharded across 8 NeuronCores (1024 rows/core); the
column operand (all 8192 embeddings) is replicated to every core, so no
collective is needed.

Device computation per core (per 128-row tile):
  - One fused matmul chain computes a NEGATED masked distance surrogate
        nval[i,j] = 2*G_ij - sq_j - 32768*[host_i==host_j]
    in PSUM via an extended contraction: fp16 hi split Eh=fp16(e) plus
    the Eh.El cross term (El = e-Eh, exact power-of-two +-64 balancing),
    G ~ Eh.Eh + Eh.El, |error| <~ 4e-3; plus 3 fp16 splits of sq_j and
    a -32768*onehot64(host) mask block (5 K-chunks total -- sized so
    the warm PE stream ~ matches the DVE span, keeping the PE dense and
    HAM-warm). sq_i omitted (row constant), sqrt omitted (monotone).
  - ACT copies PSUM -> SBUF as fp16 (frees PSUM banks).
  - DVE max8 per column group + combine: top1 = -min, top2 = runner-up.
  - DVE is_equal(val, top1) -> uint8 one-hot mask, DMA'd to the host.
Host: idx = mask.argmax(1) (exact first-index); rows whose device
margin (top1-top2) cannot certify the true argmin (fp16 copy quant
0.25 + model err) are recomputed exactly in fp32 numpy (<~100 rows);
final gather e_actv[idx].
"""

import numpy as np

import concourse.tile as tile
from concourse import bacc, mybir
from concourse.bass_utils import run_bass_kernel_spmd

N, D, H = 8192, 256, 64
N_CORES = 8
RPC = N // N_CORES          # rows per core
P = 128                     # partitions
RT = RPC // P               # row tiles per core (8)
TS = 512                    # matmul free-dim (one PSUM bank)
GW = 2048                   # column group width (4 PSUM banks)
NG = N // GW                # column groups (4)
NCHUNK = 3                  # Eh0 Eh1 extras
CTW = NCHUNK * GW
BIGM = 32768.0
SCALE = np.float32(64.0)
RESCUE_THR = 0.8            # device margin below which the host recomputes

f16 = np.float16

_compiled = None


def _build():
    nc = bacc.Bacc("TRN2", target_bir_lowering=False, debug=False,
                   num_devices=N_CORES)
    b_in = nc.dram_tensor("b_in", [NG, P, CTW], mybir.dt.float16,
                          kind="ExternalInput").ap()
    a_in = nc.dram_tensor("a_in", [RT, P, NCHUNK * P], mybir.dt.float16,
                          kind="ExternalInput").ap()
    out_mask = nc.dram_tensor("out_mask", [RPC, N], mybir.dt.uint8,
                              kind="ExternalOutput").ap()
    out_top = nc.dram_tensor("out_top", [RPC, 2], mybir.dt.float32,
                             kind="ExternalOutput").ap()

    with tile.TileContext(nc) as tc:
        with tc.tile_pool(name="bp", bufs=1) as bp, \
             tc.tile_pool(name="apool", bufs=4) as apool, \
             tc.tile_pool(name="vp", bufs=3) as vp, \
             tc.tile_pool(name="mp", bufs=2) as mp, \
             tc.tile_pool(name="small", bufs=4) as small, \
             tc.tile_pool(name="psum", bufs=2, space="PSUM") as pp:
            def load_a(rt):
                at = apool.tile([P, NCHUNK * P], mybir.dt.float16, tag="a")
                nc.sync.dma_start(at[:], a_in[rt])
                return at

            atiles = [load_a(0), load_a(1)]
            btile = bp.tile([P, NG * CTW], mybir.dt.float16, tag="b")
            for g in range(NG):
                nc.sync.dma_start(btile[:, g * CTW:(g + 1) * CTW], b_in[g])

            for rt in range(RT):
                at = atiles[rt]
                if rt + 2 < RT:
                    atiles.append(load_a(rt + 2))
                v = vp.tile([P, N], mybir.dt.float16, tag="val")
                t8 = small.tile([P, 40], mybir.dt.float32, tag="t8")
                for g in range(NG):
                    ps = pp.tile([P, GW], mybir.dt.float32, tag="ps")
                    for s in range(GW // TS):
                        for c in range(NCHUNK):
                            lhsT = at[:, c * P:(c + 1) * P]
                            boff = g * CTW + c * GW + s * TS
                            rhs = btile[:, boff:boff + TS]
                            nc.tensor.matmul(
                                ps[:, s * TS:(s + 1) * TS], lhsT, rhs,
                                start=(c == 0), stop=(c == NCHUNK - 1))
                    nc.scalar.activation(v[:, g * GW:(g + 1) * GW], ps[:],
                                         mybir.ActivationFunctionType.Copy)
                    nc.vector.max(out=t8[:, 8 + g * 8:16 + g * 8],
                                  in_=v[:, g * GW:(g + 1) * GW])

                nc.vector.max(out=t8[:, 0:8], in_=t8[:, 8:40])
                mask = mp.tile([P, N], mybir.dt.uint8, tag="mask")
                nc.vector.tensor_scalar(mask[:], v[:], t8[:, 0:1], None,
                                        op0=mybir.AluOpType.is_equal)
                r0 = rt * P
                nc.sync.dma_start(out_mask[r0:r0 + P, :], mask[:])
                nc.sync.dma_start(out_top[r0:r0 + P, :], t8[:, 0:2])

    nc.compile()
    return nc


def _prep_inputs(e_actv: np.ndarray, host: np.ndarray):
    e = np.ascontiguousarray(np.asarray(e_actv, dtype=np.float32))
    hostv = np.asarray(host).astype(np.int64)

    eh = e.astype(f16)
    ehf = eh.astype(np.float32)
    elf = e - ehf

    chunks_b = [eh]
    chunks_a = [(2.0 * ehf).astype(f16)]

    sq = (e.astype(np.float64) * e.astype(np.float64)).sum(1)
    s1 = sq.astype(np.float32).astype(f16)
    r1 = (sq - s1.astype(np.float64)).astype(np.float32)
    s2 = r1.astype(f16)
    s3 = (r1 - s2.astype(np.float32)).astype(f16)

    onehot = np.zeros((N, H), dtype=np.float32)
    onehot[np.arange(N), hostv] = 1.0

    bstack = np.zeros((NCHUNK, P, N), dtype=f16)
    for kk, cb in enumerate(chunks_b):
        cbT = np.ascontiguousarray(cb.T)
        bstack[2 * kk] = cbT[:P]
        bstack[2 * kk + 1] = cbT[P:]
    bx = np.zeros((P, N), dtype=np.float32)
    bx[0] = -s1.astype(np.float32)
    bx[1] = -s2.astype(np.float32)
    bx[2] = -s3.astype(np.float32)
    bx[3:3 + H] = onehot.T * np.float32(-BIGM)
    bstack[2] = bx.astype(f16)
    b_all = np.ascontiguousarray(
        bstack.reshape(NCHUNK, P, NG, GW).transpose(2, 1, 0, 3)
    ).reshape(NG, P, CTW)

    a_maps = []
    for core in range(N_CORES):
        a_all = np.zeros((RT, P, NCHUNK * P), dtype=f16)
        for rt in range(RT):
            r0 = core * RPC + rt * P
            rows = slice(r0, r0 + P)
            for kk, ca in enumerate(chunks_a):
                caT = np.ascontiguousarray(ca[rows].T)
                a_all[rt, :, (2 * kk) * P:(2 * kk + 1) * P] = caT[:P]
                a_all[rt, :, (2 * kk + 1) * P:(2 * kk + 2) * P] = caT[P:]
            ax = np.zeros((P, P), dtype=np.float32)
            ax[0:3, :] = 1.0
            ax[3:3 + H, :] = onehot[rows].T
            a_all[rt, :, 2 * P:3 * P] = ax.astype(f16)
        a_maps.append({"b_in": b_all, "a_in": a_all})
    return a_maps


def _run(in_maps, trace=False, **kw):
    global _compiled
    if _compiled is None:
        _compiled = _build()
    return run_bass_kernel_spmd(_compiled, in_maps, list(range(N_CORES)),
                                trace=trace, **kw)


def _exact_rows(e, hostv, rows):
    """Exact fp32 masked argmin for the given rows (reference semantics)."""
    sq = (e * e).sum(1)
    G = e[rows] @ e.T
    d2 = sq[rows][:, None] + sq[None, :] - 2.0 * G
    d2 = np.where(hostv[rows][:, None] == hostv[None, :], np.float32(1e30), d2)
    return d2.argmin(1)


def kernel(e_actv, e_ap, host):
    e = np.ascontiguousarray(np.asarray(e_actv, dtype=np.float32))
    hostv = np.asarray(host).astype(np.int64)
    in_maps = _prep_inputs(e, hostv)
    res = _run(in_maps)

    masks = np.concatenate([res.results[c]["out_mask"] for c in range(N_CORES)])
    tops = np.concatenate([res.results[c]["out_top"] for c in range(N_CORES)])
    idx = masks.argmax(axis=1)

    margin = tops[:, 0] - tops[:, 1]
    rescue = np.where((margin < RESCUE_THR) | (masks.max(axis=1) == 0))[0]
    if len(rescue):
        idx[rescue] = _exact_rows(e, hostv, rescue)

    e_an = np.asarray(e_actv)[idx]
    return (np.asarray(e_actv), np.asarray(e_ap), e_an)


# revision 14
# speedup vs baseline: 2.6254x; 1.3243x over previous
"""Masked nearest-neighbor (AnchorTs2Vec e_an) Trainium2 kernel.

Problem: for e_actv [8192, 256] f32 and host ids [8192], compute
    d2[i,j] = |e_i|^2 + |e_j|^2 - 2 e_i.e_j   (masked BIG where host_i==host_j, incl. diag)
    idx[i]  = argmin_j d2[i,j]   (first index on ties, matching jnp.argmin)
    e_an    = e_actv[idx]
Returns (e_actv, e_ap, e_an) like the reference.

Distribution: rows sharded across 8 NeuronCores (1024 rows/core); the
column operand (all 8192 embeddings) is replicated to every core, so no
collective is needed.

Device computation per core (per 128-row tile):
  - One fused matmul chain computes a NEGATED masked distance surrogate
        nval[i,j] = 2*G_ij - sq_j - 32768*[host_i==host_j]
    in PSUM via an extended contraction: fp16 hi split Eh=fp16(e) plus
    the Eh.El cross term (El = e-Eh, exact power-of-two +-64 balancing),
    G ~ Eh.Eh + Eh.El, |error| <~ 4e-3; plus 3 fp16 splits of sq_j and
    a -32768*onehot64(host) mask block (5 K-chunks total -- sized so
    the warm PE stream ~ matches the DVE span, keeping the PE dense and
    HAM-warm). sq_i omitted (row constant), sqrt omitted (monotone).
  - ACT copies PSUM -> SBUF as fp16 (frees PSUM banks).
  - DVE max8 per column group + combine: top1 = -min, top2 = runner-up.
  - DVE is_equal(val, top1) -> uint8 one-hot mask, DMA'd to the host.
Host: idx = mask.argmax(1) (exact first-index); rows whose device
margin (top1-top2) cannot certify the true argmin (fp16 copy quant
0.25 + model err) are recomputed exactly in fp32 numpy (<~100 rows);
final gather e_actv[idx].
"""

import numpy as np

import concourse.tile as tile
from concourse import bacc, mybir
from concourse.bass_utils import run_bass_kernel_spmd

N, D, H = 8192, 256, 64
N_CORES = 8
RPC = N // N_CORES          # rows per core
P = 128                     # partitions
RT = RPC // P               # row tiles per core (8)
TS = 512                    # matmul free-dim (one PSUM bank)
GW = 2048                   # column group width (4 PSUM banks)
NG = N // GW                # column groups (4)
NCHUNK = 3                  # Eh0 Eh1 extras
CTW = NCHUNK * GW
BIGM = 32768.0
SCALE = np.float32(64.0)
RESCUE_THR = 0.8            # device margin below which the host recomputes

f16 = np.float16

_compiled = None


def _build():
    nc = bacc.Bacc("TRN2", target_bir_lowering=False, debug=False,
                   num_devices=N_CORES)
    b_in = nc.dram_tensor("b_in", [NG, P, CTW], mybir.dt.float16,
                          kind="ExternalInput").ap()
    a_in = nc.dram_tensor("a_in", [RT, P, NCHUNK * P], mybir.dt.float16,
                          kind="ExternalInput").ap()
    out_mask = nc.dram_tensor("out_mask", [RPC, N], mybir.dt.float16,
                              kind="ExternalOutput").ap()

    with tile.TileContext(nc) as tc:
        with tc.tile_pool(name="bp", bufs=1) as bp, \
             tc.tile_pool(name="apool", bufs=4) as apool, \
             tc.tile_pool(name="vp", bufs=3) as vp, \
             tc.tile_pool(name="mp", bufs=2) as mp, \
             tc.tile_pool(name="small", bufs=4) as small, \
             tc.tile_pool(name="psum", bufs=2, space="PSUM") as pp:
            def load_a(rt):
                at = apool.tile([P, NCHUNK * P], mybir.dt.float16, tag="a")
                nc.sync.dma_start(at[:], a_in[rt])
                return at

            atiles = [load_a(0), load_a(1)]
            btile = bp.tile([P, NG * CTW], mybir.dt.float16, tag="b")
            for g in range(NG):
                nc.sync.dma_start(btile[:, g * CTW:(g + 1) * CTW], b_in[g])

            for rt in range(RT):
                at = atiles[rt]
                if rt + 2 < RT:
                    atiles.append(load_a(rt + 2))
                v = vp.tile([P, N], mybir.dt.float16, tag="val")
                tg = small.tile([P, 4], mybir.dt.float32, tag="tg")
                for g in range(NG):
                    ps = pp.tile([P, GW], mybir.dt.float32, tag="ps")
                    for s in range(GW // TS):
                        for c in range(NCHUNK):
                            lhsT = at[:, c * P:(c + 1) * P]
                            boff = g * CTW + c * GW + s * TS
                            rhs = btile[:, boff:boff + TS]
                            nc.tensor.matmul(
                                ps[:, s * TS:(s + 1) * TS], lhsT, rhs,
                                start=(c == 0), stop=(c == NCHUNK - 1))
                    nc.scalar.activation(v[:, g * GW:(g + 1) * GW], ps[:],
                                         mybir.ActivationFunctionType.Copy)
                    nc.vector.tensor_reduce(tg[:, g:g + 1],
                                            v[:, g * GW:(g + 1) * GW],
                                            axis=mybir.AxisListType.X,
                                            op=mybir.AluOpType.max)

                t1 = small.tile([P, 1], mybir.dt.float32, tag="t1")
                nc.vector.tensor_reduce(t1[:], tg[:], axis=mybir.AxisListType.X,
                                        op=mybir.AluOpType.max)
                thr = small.tile([P, 1], mybir.dt.float32, tag="thr")
                nc.vector.tensor_scalar(thr[:], t1[:], -RESCUE_THR, None,
                                        op0=mybir.AluOpType.add)
                mask = mp.tile([P, N], mybir.dt.float16, tag="mask")
                nc.vector.tensor_scalar(mask[:], v[:], thr[:, 0:1], None,
                                        op0=mybir.AluOpType.is_ge)
                r0 = rt * P
                nc.sync.dma_start(out_mask[r0:r0 + P, :], mask[:])

    nc.compile()
    return nc


def _prep_inputs(e_actv: np.ndarray, host: np.ndarray):
    e = np.ascontiguousarray(np.asarray(e_actv, dtype=np.float32))
    hostv = np.asarray(host).astype(np.int64)

    eh = e.astype(f16)
    ehf = eh.astype(np.float32)
    elf = e - ehf

    chunks_b = [eh]
    chunks_a = [(2.0 * ehf).astype(f16)]

    sq = (e.astype(np.float64) * e.astype(np.float64)).sum(1)
    s1 = sq.astype(np.float32).astype(f16)
    r1 = (sq - s1.astype(np.float64)).astype(np.float32)
    s2 = r1.astype(f16)
    s3 = (r1 - s2.astype(np.float32)).astype(f16)

    onehot = np.zeros((N, H), dtype=np.float32)
    onehot[np.arange(N), hostv] = 1.0

    bstack = np.zeros((NCHUNK, P, N), dtype=f16)
    for kk, cb in enumerate(chunks_b):
        cbT = np.ascontiguousarray(cb.T)
        bstack[2 * kk] = cbT[:P]
        bstack[2 * kk + 1] = cbT[P:]
    bx = np.zeros((P, N), dtype=np.float32)
    bx[0] = -s1.astype(np.float32)
    bx[1] = -s2.astype(np.float32)
    bx[2] = -s3.astype(np.float32)
    bx[3:3 + H] = onehot.T * np.float32(-BIGM)
    bstack[2] = bx.astype(f16)
    b_all = np.ascontiguousarray(
        bstack.reshape(NCHUNK, P, NG, GW).transpose(2, 1, 0, 3)
    ).reshape(NG, P, CTW)

    a_maps = []
    for core in range(N_CORES):
        a_all = np.zeros((RT, P, NCHUNK * P), dtype=f16)
        for rt in range(RT):
            r0 = core * RPC + rt * P
            rows = slice(r0, r0 + P)
            for kk, ca in enumerate(chunks_a):
                caT = np.ascontiguousarray(ca[rows].T)
                a_all[rt, :, (2 * kk) * P:(2 * kk + 1) * P] = caT[:P]
                a_all[rt, :, (2 * kk + 1) * P:(2 * kk + 2) * P] = caT[P:]
            ax = np.zeros((P, P), dtype=np.float32)
            ax[0:3, :] = 1.0
            ax[3:3 + H, :] = onehot[rows].T
            a_all[rt, :, 2 * P:3 * P] = ax.astype(f16)
        a_maps.append({"b_in": b_all, "a_in": a_all})
    return a_maps


def _run(in_maps, trace=False, **kw):
    global _compiled
    if _compiled is None:
        _compiled = _build()
    return run_bass_kernel_spmd(_compiled, in_maps, list(range(N_CORES)),
                                trace=trace, **kw)


def _exact_rows(e, hostv, rows):
    """Exact fp32 masked argmin for the given rows (reference semantics)."""
    sq = (e * e).sum(1)
    G = e[rows] @ e.T
    d2 = sq[rows][:, None] + sq[None, :] - 2.0 * G
    d2 = np.where(hostv[rows][:, None] == hostv[None, :], np.float32(1e30), d2)
    return d2.argmin(1)


def kernel(e_actv, e_ap, host):
    e = np.ascontiguousarray(np.asarray(e_actv, dtype=np.float32))
    hostv = np.asarray(host).astype(np.int64)
    in_maps = _prep_inputs(e, hostv)
    res = _run(in_maps)

    masks = np.concatenate([res.results[c]["out_mask"] for c in range(N_CORES)])
    marked = masks > 0
    idx = marked.argmax(axis=1)
    rescue = np.where(marked.sum(axis=1) != 1)[0]
    if len(rescue):
        idx[rescue] = _exact_rows(e, hostv, rescue)

    e_an = np.asarray(e_actv)[idx]
    return (np.asarray(e_actv), np.asarray(e_ap), e_an)
